# revision 2
# baseline (speedup 1.0000x reference)
"""Causal self-attention (B=2, T=2048, EMB=1024, 16 heads) on 8 TRN2 NeuronCores.

Sharding: core c handles batch c//4 and heads [4*(c%4), 4*(c%4)+4).
 - Wqkv is split column-wise per head group (q part pre-scaled by 1/sqrt(hd)),
 - Wproj is split row-wise per head group,
 - each core emits a partial [2048, 1024] projection output,
 - host sums the 4 partials per batch and adds bproj (row-parallel unshard).

Device kernel (per core, SPMD):
 - host supplies x^T so both qkv matmul operands have the contraction on
   partitions; qkT is produced directly in [qkv_col, token] (transposed) layout.
 - v is produced transposed too, then PE-transposed to token-major and
   augmented with a ones column (row-sum trick for the softmax denominator).
 - attention runs in the S^T = (K Q^T) layout, chunk-major: for each output
   token chunk of 512, for each head pair, accumulate P^T V into a [65, 512]
   PSUM tile per head (row 64 = softmax denominator via the ones column).
 - both heads of a pair write their S^T j-tile into one [128, 2, 512] f32
   PSUM tile so ONE exp activation covers the pair (halves ACT instruction
   count); the causal-diagonal mask is one merged [128, 2, 128] multiply.
 - the softmax denominator reciprocal is a single DVE op on the [1, 512]
   PSUM row (no partition-spread DMA round trips on the ACT queue).
 - PE-program-order stalls are avoided by deferring every PE-touching epilogue
   (reciprocal broadcast matmul, normalize, projection) two units behind the
   attention loop that produces its inputs.

All matmuls run in f16 operands (~1e-4 rel err); PSUM accumulation in f32.
"""
import sys

sys.path.insert(0, "/opt/trn_rl_repo")

import numpy as np

B = 2
T = 2048
EMB = 1024
HEADS = 16
HD = EMB // HEADS  # 64
NCORES = 8
GROUPS = 4                 # head groups (cores per batch)
HPC = HEADS // GROUPS      # 4 heads per core
CQ = HPC * HD              # 256 q (or k or v) columns per core
NCB = 3 * CQ // 128        # 6 col-tiles of 128 in the qkv projection
KT = EMB // 128            # 8 contraction tiles
TCH = 512                  # token chunk
NCH = T // TCH             # 4 chunks
NTT = T // 128             # 16 token tiles
NR = CQ // 128             # 2 head-dim row tiles
SCALE = HD ** -0.5

_compiled = {}
ABLATE = None  # None | 's1' (stage1 only) | 's12' (no projection)


def _build(loop=1):
    import concourse.bass as bass
    import concourse.tile as tile
    from concourse import bacc, mybir
    from concourse.masks import make_identity

    F32 = mybir.dt.float32
    F16 = mybir.dt.float16
    AF = mybir.ActivationFunctionType

    nc = bacc.Bacc(None, target_bir_lowering=False)
    xT = nc.dram_tensor("xT", [EMB, T], F16, kind="ExternalInput")
    wqkv = nc.dram_tensor("wqkv", [EMB, 3 * CQ], F16, kind="ExternalInput")
    bqkv = nc.dram_tensor("bqkv", [128, NCB], F32, kind="ExternalInput")
    wproj = nc.dram_tensor("wproj", [CQ, EMB], F16, kind="ExternalInput")
    out = nc.dram_tensor("out", [T, EMB], F32, kind="ExternalOutput")

    xT_r = xT.rearrange("(kt p) t -> p kt t", p=128)
    wqkv_r = wqkv.rearrange("(kt p) c -> p kt c", p=128)
    wproj_r = wproj.rearrange("(r p) e -> p r e", p=128)

    with tile.TileContext(nc) as tc:
        with (
            tc.tile_pool(name="const", bufs=1) as const,
            tc.tile_pool(name="qk", bufs=1) as qkp,
            tc.tile_pool(name="xt", bufs=3) as xtp,
            tc.tile_pool(name="vt", bufs=2) as vtp,
            tc.tile_pool(name="pt", bufs=4) as ptp,
            tc.tile_pool(name="oh", bufs=1) as ohp,
            tc.tile_pool(name="den", bufs=5) as denp,
            tc.tile_pool(name="osb", bufs=3) as osbp,
            tc.tile_pool(name="ps", bufs=2, space="PSUM") as psS,
            tc.tile_pool(name="psO", bufs=4, space="PSUM") as psO,
        ):
            # ---- constants ----
            # weights on the scalar HWDGE queue, per k-tile, so the sync
            # queue's xt chunk loads run in parallel and matmuls start early
            bias_sb = const.tile([128, NCB], F32)
            nc.scalar.dma_start(out=bias_sb, in_=bqkv[:, :])
            w_sb = const.tile([128, KT, 3 * CQ], F16)
            for kt in range(KT):
                nc.scalar.dma_start(
                    out=w_sb[:, kt, 0:CQ], in_=wqkv_r[:, kt, 0:CQ]
                )
            for cp in range(1, 3):
                nc.scalar.dma_start(
                    out=w_sb[:, :, cp * CQ : (cp + 1) * CQ],
                    in_=wqkv_r[:, :, cp * CQ : (cp + 1) * CQ],
                )
            # stage-3 weights loaded inside body() after the xt chunks
            wp_sb = const.tile([128, NR, EMB], F16)
            ident = const.tile([128, 128], F16)
            make_identity(nc, ident)
            tri_f = const.tile([128, 128], F32)
            nc.gpsimd.memset(tri_f, 1.0)
            # keep where i(free) >= j(partition): -j + i >= 0
            nc.gpsimd.affine_select(
                out=tri_f, in_=tri_f,
                compare_op=mybir.AluOpType.is_ge,
                fill=0.0, base=0,
                pattern=[[1, 128]], channel_multiplier=-1,
            )
            # causal mask replicated for both heads of a pair
            tri2 = const.tile([128, 2, 128], F16)
            nc.vector.tensor_copy(tri2[:, 0, :], tri_f)
            nc.vector.tensor_copy(tri2[:, 1, :], tri_f)
            ones_f = const.tile([128, 64], F32)
            nc.vector.memset(ones_f, 1.0)
            ones64 = const.tile([1, 64], F16)
            nc.vector.tensor_copy(ones64, ones_f[0:1, :])
            # v in token-major, per (token_tile, head): 64 cols + ones col
            v_sb = const.tile([128, NTT, HPC, HD + 1], F16)
            nc.vector.tensor_copy(
                out=v_sb[:, :, :, HD : HD + 1],
                in_=ones_f.rearrange("p (a b c) -> p a b c", a=NTT, b=HPC),
            )
            qkT_sb = qkp.tile([128, 4, T], F16)
            ohT = ohp.tile([128, NR, T], F16)

            def body():
                # ---- stage 1: qkv projection (transposed layout) ----
                # qkT_sb[:, cb, t]: cb 0,1 = q col-tiles, 2,3 = k col-tiles
                def stage1_chunk(ch):
                    xt = xtp.tile([128, KT, TCH], F16)
                    if ch == 0:
                        for kt in range(KT):
                            nc.sync.dma_start(
                                out=xt[:, kt, :],
                                in_=xT_r[:, kt, ch * TCH : (ch + 1) * TCH],
                            )
                    else:
                        nc.sync.dma_start(
                            out=xt, in_=xT_r[:, :, ch * TCH : (ch + 1) * TCH]
                        )
                    for cb in range(NCB):
                        ps = psS.tile([128, TCH], mybir.dt.float32, tag="ps")
                        for kt in range(KT):
                            nc.tensor.matmul(
                                ps,
                                w_sb[:, kt, cb * 128 : (cb + 1) * 128],
                                xt[:, kt, :],
                                start=(kt == 0),
                                stop=(kt == KT - 1),
                            )
                        if cb < 4:
                            nc.vector.tensor_scalar_add(
                                qkT_sb[:, cb, ch * TCH : (ch + 1) * TCH],
                                ps,
                                bias_sb[:, cb : cb + 1],
                            )
                        else:
                            vt = vtp.tile([128, TCH], F16)
                            nc.vector.tensor_scalar_add(
                                vt, ps, bias_sb[:, cb : cb + 1]
                            )
                            for s in range(TCH // 128):
                                tt = ch * (TCH // 128) + s
                                tp = psS.tile([128, TCH], F16, tag="ps")
                                nc.tensor.transpose(
                                    tp[:, 0:128], vt[:, s * 128 : (s + 1) * 128], ident
                                )
                                for hh in range(2):
                                    h = 2 * (cb - 4) + hh
                                    nc.vector.tensor_copy(
                                        v_sb[:, tt, h, 0:HD],
                                        tp[:, hh * HD : (hh + 1) * HD],
                                    )

                # ---- stages 2+3: chunk-major attention + streaming epilogue ----
                pending = []  # (unit_idx, emit_fn): PE-touching epilogues, lag 2

                def flush(upto):
                    while pending and pending[0][0] <= upto:
                        pending.pop(0)[1]()

                def make_partB(psC2, rec_rows, r, base):
                    def partB():
                        # packed reciprocal broadcast: head-even -> psum rows
                        # 0:64 (col group 0), head-odd -> rows 64:128 (col
                        # group 64); they run concurrently on the PE
                        rp = psS.tile([128, TCH], mybir.dt.float32, tag="ps")
                        nc.tensor.matmul(
                            rp[0:64, :], ones64, rec_rows[0], start=True, stop=True
                        )
                        nc.tensor.matmul(
                            rp[64:128, :], ones64, rec_rows[1], start=True, stop=True
                        )
                        rec_sb = denp.tile([128, TCH], F32, tag="rec_sb")
                        nc.vector.tensor_copy(rec_sb, rp)
                        nc.vector.tensor_mul(
                            ohT[0:64, r, base : base + TCH],
                            psC2[0][0:64, :],
                            rec_sb[0:64, :],
                        )
                        nc.vector.tensor_mul(
                            ohT[64:128, r, base : base + TCH],
                            psC2[1][0:64, :],
                            rec_sb[64:128, :],
                        )
                    return partB

                def make_proj(cc):
                    def proj():
                        for tt in range(4 * cc, 4 * cc + 4):
                            for nn in range(EMB // 512):
                                pp = psS.tile(
                                    [128, TCH], mybir.dt.float32, tag="ps"
                                )
                                for r2 in range(NR):
                                    nc.tensor.matmul(
                                        pp,
                                        ohT[:, r2, tt * 128 : (tt + 1) * 128],
                                        wp_sb[:, r2, nn * 512 : (nn + 1) * 512],
                                        start=(r2 == 0),
                                        stop=(r2 == NR - 1),
                                    )
                                osb = osbp.tile([128, 512], F32)
                                nc.any.tensor_copy(osb, pp)
                                nc.sync.dma_start(
                                    out=out[
                                        tt * 128 : (tt + 1) * 128,
                                        nn * 512 : (nn + 1) * 512,
                                    ],
                                    in_=osb,
                                )
                    return proj

                unit = 0

                def emit_unit(cc, r):
                    # one unit = head pair (2r, 2r+1): both heads' S^T j-tiles
                    # land in one [128, 2, TCH] PSUM tile -> single exp each
                    nonlocal unit
                    base = cc * TCH
                    jmax = 4 * cc + 3
                    psC2 = []
                    for _h in range(2):
                        psC_t = psO.tile([65, TCH], mybir.dt.float32, tag="psO")
                        psC2.append(psC_t)
                    diag = [j for j in range(4 * cc, jmax + 1) if j != 0]
                    rest = [j for j in range(1, 4 * cc)]
                    order = [0] + diag + rest
                    flush_pos = min(3, len(order) - 1)
                    prev = None  # PV lags S/exp by one j-tile
                    for pos, jt in enumerate(order):
                        i0 = 128 * jt
                        lo = max(base, i0)
                        hi = base + TCH
                        w = hi - lo
                        sp = psS.tile([128, 2, TCH], mybir.dt.float32, tag="ps")
                        for hh in range(2):
                            po = 64 * hh
                            nc.tensor.matmul(
                                sp[:, hh, 0:w],
                                qkT_sb[po : po + 64, 2 + r, i0 : i0 + 128],
                                qkT_sb[po : po + 64, r, lo:hi],
                                start=True,
                                stop=True,
                            )
                        pt = ptp.tile([128, 2, TCH], F16)
                        nc.scalar.activation(pt[:, :, 0:w], sp[:, :, 0:w], AF.Exp)
                        if i0 >= base:  # diagonal block: causal mask
                            nc.vector.tensor_mul(
                                pt[:, :, 0:128], pt[:, :, 0:128], tri2
                            )
                        if pos == flush_pos:
                            flush(unit - 1)
                        if prev is not None:
                            _emit_pv_pair(nc, psC2, v_sb, prev, r, base, order[-1])
                        prev = (jt, pt, lo, hi)
                    _emit_pv_pair(nc, psC2, v_sb, prev, r, base, order[-1])

                    # part A: denominator reciprocal, one DVE op per head
                    rec_rows = []
                    for hh in range(2):
                        rec_row = denp.tile([1, TCH], F16, tag="rec_row")
                        with nc.allow_low_precision(
                            reason="rec broadcast operand is f16 by design"
                        ):
                            nc.vector.reciprocal(rec_row, psC2[hh][64:65, :])
                        rec_rows.append(rec_row)
                    pending.append((unit, make_partB(psC2, rec_rows, r, base)))
                    if r == NR - 1 and ABLATE != "s12":
                        pending.append((unit, make_proj(cc)))
                    unit += 1

                # interleave: attention units for chunk cc are emitted as soon
                # as stage-1 chunks 0..cc exist, so PE never starves on either
                # the stage-1 DMA feed or the attention epilogue latency
                stage1_chunk(0)
                stage1_chunk(1)
                if ABLATE == "s1":
                    stage1_chunk(2)
                    stage1_chunk(3)
                    return
                for r in range(NR):
                    emit_unit(0, r)
                # stage-3 weights: on the sync queue behind xt0/xt1, ready
                # well before proj(0) is flushed (re-loaded per loop iter)
                nc.sync.dma_start(out=wp_sb, in_=wproj_r)
                stage1_chunk(2)
                for r in range(NR):
                    emit_unit(1, r)
                stage1_chunk(3)
                for r in range(NR):
                    emit_unit(2, r)
                for r in range(NR):
                    emit_unit(3, r)
                flush(unit)

            if loop == 1:
                body()
            else:
                with tc.For_i(
                    0, loop, 1,
                    hint_engines=(
                        mybir.EngineType.PE,
                        mybir.EngineType.Activation,
                        mybir.EngineType.DVE,
                        mybir.EngineType.SP,
                        mybir.EngineType.Pool,
                    ),
                ):
                    body()

    nc.finalize()
    return nc


def _emit_pv_pair(nc, psC2, v_sb, prev, r, base, jlast):
    """P^T[jt] @ v_aug for both heads of the pair, accumulated into their
    chunk PSUM tiles."""
    jt, pt, lo, hi = prev
    for hh in range(2):
        nc.tensor.matmul(
            psC2[hh][:, lo - base : hi - base],
            v_sb[:, jt, 2 * r + hh, :],
            pt[:, hh, 0 : hi - lo],
            start=(jt == 0),
            stop=(jt == jlast),
            skip_group_check=(jt != 0),
        )


def _shard_inputs(x, Wqkv, bqkv, Wproj):
    """Build the 8 per-core input maps."""
    x = np.asarray(x, dtype=np.float32)
    Wqkv = np.asarray(Wqkv, dtype=np.float32)
    bqkv = np.asarray(bqkv, dtype=np.float32)
    Wproj = np.asarray(Wproj, dtype=np.float32)

    in_maps = []
    for c in range(NCORES):
        b = c // GROUPS
        g = c % GROUPS
        cols = slice(g * CQ, (g + 1) * CQ)
        wq = Wqkv[:, cols] * SCALE
        wk = Wqkv[:, EMB:][:, cols]
        wv = Wqkv[:, 2 * EMB:][:, cols]
        w_c = np.ascontiguousarray(
            np.concatenate([wq, wk, wv], axis=1).astype(np.float16)
        )
        bq = bqkv[cols] * SCALE
        bk = bqkv[EMB:][cols]
        bv = bqkv[2 * EMB:][cols]
        b_c = np.concatenate([bq, bk, bv])  # [768]
        b_c = np.ascontiguousarray(b_c.reshape(NCB, 128).T)  # [128, 6]
        wp_c = np.ascontiguousarray(Wproj[cols, :].astype(np.float16))
        xT_c = np.ascontiguousarray(x[b].T.astype(np.float16))  # [1024, 2048]
        in_maps.append({"xT": xT_c, "wqkv": w_c, "bqkv": b_c, "wproj": wp_c})
    return in_maps


def run(inputs, trace=False, **kwargs):
    """Build (cached), run on 8 cores, return (full_output, BassKernelResults)."""
    from concourse.bass_utils import run_bass_kernel_spmd

    if _compiled.get(1) is None:
        _compiled[1] = _build()
    in_maps = _shard_inputs(
        inputs["x"], inputs["Wqkv"], inputs["bqkv"], inputs["Wproj"]
    )
    res = run_bass_kernel_spmd(
        _compiled[1], in_maps, core_ids=list(range(NCORES)), trace=trace, **kwargs
    )
    partials = np.stack([res.results[c]["out"] for c in range(NCORES)])  # [8,T,EMB]
    bproj = np.asarray(inputs["bproj"], dtype=np.float32)
    full = np.stack(
        [partials[b * GROUPS : (b + 1) * GROUPS].sum(axis=0) for b in range(B)]
    ) + bproj
    return full.astype(np.float32), res


def kernel(**inputs):
    out, _ = run(inputs)
    return out


# revision 3
# speedup vs baseline: 1.1384x; 1.1384x over previous
"""Causal self-attention (B=2, T=2048, EMB=1024, 16 heads) on 8 TRN2 NeuronCores.

Sharding: core c handles batch c//4 and heads [4*(c%4), 4*(c%4)+4).
 - Wqkv is split column-wise per head group (q part pre-scaled by 1/sqrt(hd)),
 - Wproj is split row-wise per head group,
 - each core emits a partial [2048, 1024] projection output,
 - host sums the 4 partials per batch and adds bproj + bv@Wproj
   (softmax rows sum to 1, so the v bias contributes a constant row vector
   that the host can add; the device kernel drops bv entirely).

Device kernel (per core, SPMD):
 - host supplies x^T so both qkv matmul operands have the contraction on
   partitions; qkT is produced directly in [qkv_col, token] (transposed)
   layout. v is produced token-major directly (lhsT = x^T token tile), no
   PE transposes needed, and lands next to a ones column (row-sum trick
   for the softmax denominator).
 - attention runs in the S^T = (K Q^T) layout, chunk-major: for each output
   token chunk of 512, for each head, accumulate P^T V into a [65, 512] PSUM
   tile (row 64 = softmax denominator). PV matmuls are emitted as contiguous
   per-head 4-chains (lagging one block of 4 j-tiles) to avoid the per-matmul
   PSUM-group-switch cost the HW charges on alternating accumulation targets.
 - softmax denominator DMAs (partition spread for the 128-lane reciprocal)
   ride the gpsimd/Pool SWDGE queue so they never block the activation queue.
 - PE-program-order stalls are avoided by deferring every PE-touching epilogue
   (reciprocal broadcast matmul, normalize, projection) two units behind the
   attention loop that produces its inputs.

All matmul operands are f16 (~1e-4 rel err); PSUM accumulation in f32.
"""
import sys

sys.path.insert(0, "/opt/trn_rl_repo")

import numpy as np

B = 2
T = 2048
EMB = 1024
HEADS = 16
HD = EMB // HEADS  # 64
NCORES = 8
GROUPS = 4                 # head groups (cores per batch)
HPC = HEADS // GROUPS      # 4 heads per core
CQ = HPC * HD              # 256 q (or k or v) columns per core
NCB = 3 * CQ // 128        # 6 col-tiles of 128 in the qkv projection
KT = EMB // 128            # 8 contraction tiles
TCH = 512                  # token chunk
NCH = T // TCH             # 4 chunks
NTT = T // 128             # 16 token tiles
NR = CQ // 128             # 2 head-dim row tiles
SCALE = HD ** -0.5

_compiled = {}
ABLATE = None  # None | 's1' (stage1 only) | 's12' (no projection)


def _build(loop=1):
    import concourse.bass as bass
    import concourse.tile as tile
    from concourse import bacc, mybir

    F32 = mybir.dt.float32
    F16 = mybir.dt.float16
    AF = mybir.ActivationFunctionType

    nc = bacc.Bacc(None, target_bir_lowering=False)
    xT = nc.dram_tensor("xT", [EMB, T], F16, kind="ExternalInput")
    wqkv = nc.dram_tensor("wqkv", [EMB, 3 * CQ], F16, kind="ExternalInput")
    bqkv = nc.dram_tensor("bqkv", [128, 4], F32, kind="ExternalInput")
    wproj = nc.dram_tensor("wproj", [CQ, EMB], F16, kind="ExternalInput")
    out = nc.dram_tensor("out", [T, EMB], F32, kind="ExternalOutput")

    xT_r = xT.rearrange("(kt p) t -> p kt t", p=128)
    wqkv_r = wqkv.rearrange("(kt p) c -> p kt c", p=128)
    wproj_r = wproj.rearrange("(r p) e -> p r e", p=128)

    with tile.TileContext(nc) as tc:
        with (
            tc.tile_pool(name="const", bufs=1) as const,
            tc.tile_pool(name="qk", bufs=1) as qkp,
            tc.tile_pool(name="xt", bufs=3) as xtp,
            tc.tile_pool(name="pt", bufs=16) as ptp,
            tc.tile_pool(name="oh", bufs=1) as ohp,
            tc.tile_pool(name="den", bufs=5) as denp,
            tc.tile_pool(name="osb", bufs=3) as osbp,
            tc.tile_pool(name="ps", bufs=4, space="PSUM") as psS,
            tc.tile_pool(name="psO", bufs=4, space="PSUM") as psO,
        ):
            # ---- constants ----
            # weights on the scalar HWDGE queue, per k-tile, so the sync
            # queue's xt chunk loads run in parallel and matmuls start early
            bias_sb = const.tile([128, 4], F32)
            nc.scalar.dma_start(out=bias_sb, in_=bqkv[:, :])
            w_sb = const.tile([128, KT, 3 * CQ], F16)
            for kt in range(KT):
                nc.scalar.dma_start(
                    out=w_sb[:, kt, 0:CQ], in_=wqkv_r[:, kt, 0:CQ]
                )
            for cp in range(1, 3):
                nc.scalar.dma_start(
                    out=w_sb[:, :, cp * CQ : (cp + 1) * CQ],
                    in_=wqkv_r[:, :, cp * CQ : (cp + 1) * CQ],
                )
            # stage-3 weights loaded inside body() after the xt chunks
            wp_sb = const.tile([128, NR, EMB], F16)
            tri_f = const.tile([128, 128], F32)
            nc.gpsimd.memset(tri_f, 1.0)
            # keep where i(free) >= j(partition): -j + i >= 0
            nc.gpsimd.affine_select(
                out=tri_f, in_=tri_f,
                compare_op=mybir.AluOpType.is_ge,
                fill=0.0, base=0,
                pattern=[[1, 128]], channel_multiplier=-1,
            )
            tri = const.tile([128, 128], F16)
            nc.vector.tensor_copy(tri, tri_f)
            ones_f = const.tile([128, 64], F32)
            nc.vector.memset(ones_f, 1.0)
            ones64 = const.tile([1, 64], F16)
            nc.vector.tensor_copy(ones64, ones_f[0:1, :])
            # v in token-major, per (token_tile, head): 64 cols + ones col
            v_sb = const.tile([128, NTT, HPC, HD + 1], F16)
            nc.vector.tensor_copy(
                out=v_sb[:, :, :, HD : HD + 1],
                in_=ones_f.rearrange("p (a b c) -> p a b c", a=NTT, b=HPC),
            )
            qkT_sb = qkp.tile([128, 4, T], F16)
            ohT = ohp.tile([128, NR, T], F16)

            def body():
                # ---- stage 1: qkv projection ----
                # qkT_sb[:, cb, t]: cb 0,1 = q col-tiles, 2,3 = k col-tiles
                # (transposed layout); v goes token-major straight into v_sb
                def stage1_chunk(ch):
                    xt = xtp.tile([128, KT, TCH], F16)
                    if ch == 0:
                        for kt in range(KT):
                            nc.sync.dma_start(
                                out=xt[:, kt, :],
                                in_=xT_r[:, kt, ch * TCH : (ch + 1) * TCH],
                            )
                    else:
                        nc.sync.dma_start(
                            out=xt, in_=xT_r[:, :, ch * TCH : (ch + 1) * TCH]
                        )
                    for cb in range(4):
                        ps = psS.tile([128, TCH], mybir.dt.float32, tag="ps")
                        for kt in range(KT):
                            nc.tensor.matmul(
                                ps,
                                w_sb[:, kt, cb * 128 : (cb + 1) * 128],
                                xt[:, kt, :],
                                start=(kt == 0),
                                stop=(kt == KT - 1),
                            )
                        nc.vector.tensor_scalar_add(
                            qkT_sb[:, cb, ch * TCH : (ch + 1) * TCH],
                            ps,
                            bias_sb[:, cb : cb + 1],
                        )
                    for s in range(TCH // 128):
                        tt = ch * (TCH // 128) + s
                        psv = psS.tile([128, CQ], mybir.dt.float32, tag="ps")
                        for kt in range(KT):
                            nc.tensor.matmul(
                                psv,
                                xt[:, kt, s * 128 : (s + 1) * 128],
                                w_sb[:, kt, 2 * CQ : 3 * CQ],
                                start=(kt == 0),
                                stop=(kt == KT - 1),
                            )
                        nc.vector.tensor_copy(
                            v_sb[:, tt, :, 0:HD],
                            psv.rearrange("p (h d) -> p h d", h=HPC),
                        )

                # ---- stages 2+3: chunk-major attention + streaming epilogue ----
                pending = []  # (unit_idx, emit_fn): PE-touching epilogues, lag 2

                def flush(upto):
                    while pending and pending[0][0] <= upto:
                        pending.pop(0)[1]()

                def make_partB(psC2, rec_rows, r, base):
                    def partB():
                        # packed reciprocal broadcast: head-even -> psum rows
                        # 0:64 (col group 0), head-odd -> rows 64:128 (col
                        # group 64)
                        rp = psS.tile([128, TCH], mybir.dt.float32, tag="ps")
                        nc.tensor.matmul(
                            rp[0:64, :], ones64, rec_rows[0], start=True, stop=True
                        )
                        nc.tensor.matmul(
                            rp[64:128, :], ones64, rec_rows[1], start=True, stop=True
                        )
                        rec_sb = denp.tile([128, TCH], F32, tag="rec_sb")
                        nc.vector.tensor_copy(rec_sb, rp)
                        nc.vector.tensor_mul(
                            ohT[0:64, r, base : base + TCH],
                            psC2[0][0:64, :],
                            rec_sb[0:64, :],
                        )
                        nc.vector.tensor_mul(
                            ohT[64:128, r, base : base + TCH],
                            psC2[1][0:64, :],
                            rec_sb[64:128, :],
                        )
                    return partB

                def make_proj(cc):
                    def proj():
                        for tt in range(4 * cc, 4 * cc + 4):
                            for nn in range(EMB // 512):
                                pp = psS.tile(
                                    [128, TCH], mybir.dt.float32, tag="ps"
                                )
                                for r2 in range(NR):
                                    nc.tensor.matmul(
                                        pp,
                                        ohT[:, r2, tt * 128 : (tt + 1) * 128],
                                        wp_sb[:, r2, nn * 512 : (nn + 1) * 512],
                                        start=(r2 == 0),
                                        stop=(r2 == NR - 1),
                                    )
                                osb = osbp.tile([128, 512], F32)
                                nc.any.tensor_copy(osb, pp)
                                nc.sync.dma_start(
                                    out=out[
                                        tt * 128 : (tt + 1) * 128,
                                        nn * 512 : (nn + 1) * 512,
                                    ],
                                    in_=osb,
                                )
                    return proj

                unit = 0

                def emit_unit(cc, r):
                    # one unit = head pair (2r, 2r+1); S/exp stream per j-tile
                    # while PV matmuls trail one block of 4 j-tiles behind as
                    # contiguous per-head accumulation chains
                    nonlocal unit
                    base = cc * TCH
                    jmax = 4 * cc + 3
                    psC2 = []
                    for _h in range(2):
                        psC_t = psO.tile([65, TCH], mybir.dt.float32, tag="psO")
                        psC2.append(psC_t)
                    diag = [j for j in range(4 * cc, jmax + 1) if j != 0]
                    rest = [j for j in range(1, 4 * cc)]
                    order = [0] + diag + rest
                    jlast = order[-1]
                    flush_pos = min(3, len(order) - 1)

                    def pv_chain(blk):
                        for hh in range(2):
                            for jt, pt, lo, hi in blk:
                                nc.tensor.matmul(
                                    psC2[hh][:, lo - base : hi - base],
                                    v_sb[:, jt, 2 * r + hh, :],
                                    pt[hh][:, 0 : hi - lo],
                                    start=(jt == 0),
                                    stop=(jt == jlast),
                                    skip_group_check=(jt != 0),
                                )

                    blk = []
                    for pos, jt in enumerate(order):
                        i0 = 128 * jt
                        lo = max(base, i0)
                        hi = base + TCH
                        w = hi - lo
                        pts = []
                        for hh in range(2):
                            po = 64 * hh
                            sp = psS.tile([128, TCH], mybir.dt.float32, tag="ps")
                            nc.tensor.matmul(
                                sp[:, 0:w],
                                qkT_sb[po : po + 64, 2 + r, i0 : i0 + 128],
                                qkT_sb[po : po + 64, r, lo:hi],
                                start=True,
                                stop=True,
                            )
                            pt = ptp.tile([128, TCH], F16)
                            nc.scalar.activation(pt[:, 0:w], sp[:, 0:w], AF.Exp)
                            if i0 >= base:  # diagonal block: causal mask
                                nc.vector.tensor_mul(
                                    pt[:, 0:128], pt[:, 0:128], tri
                                )
                            pts.append(pt)
                        if pos == flush_pos:
                            flush(unit - 1)
                        blk.append((jt, pts, lo, hi))
                        if len(blk) == 4 and pos < len(order) - 1:
                            pv_chain(blk)
                            blk = []
                    pv_chain(blk)

                    # part A: denominator chains (no PE instructions); the
                    # partition-spread DMAs ride the idle Pool SWDGE queue
                    rec_rows = []
                    for hh in range(2):
                        den_row = denp.tile([1, TCH], F32, tag="den_row")
                        nc.vector.tensor_copy(den_row, psC2[hh][64:65, :])
                        den128 = denp.tile([128, TCH // 128], F32, tag="den128")
                        nc.gpsimd.dma_start(out=den128, in_=den_row)
                        rec128 = denp.tile([128, TCH // 128], F32, tag="rec128")
                        nc.vector.reciprocal(rec128, den128)
                        rec16 = denp.tile([128, TCH // 128], F16, tag="rec16")
                        nc.vector.tensor_copy(rec16, rec128)
                        rec_row = denp.tile([1, TCH], F16, tag="rec_row")
                        nc.gpsimd.dma_start(out=rec_row, in_=rec16)
                        rec_rows.append(rec_row)
                    pending.append((unit, make_partB(psC2, rec_rows, r, base)))
                    if r == NR - 1 and ABLATE != "s12":
                        pending.append((unit, make_proj(cc)))
                    unit += 1

                # interleave: attention units for chunk cc are emitted as soon
                # as stage-1 chunks 0..cc exist, so PE never starves on either
                # the stage-1 DMA feed or the attention epilogue latency
                stage1_chunk(0)
                stage1_chunk(1)
                if ABLATE == "s1":
                    stage1_chunk(2)
                    stage1_chunk(3)
                    return
                for r in range(NR):
                    emit_unit(0, r)
                # stage-3 weights: on the sync queue behind xt0/xt1, ready
                # well before proj(0) is flushed (re-loaded per loop iter)
                nc.sync.dma_start(out=wp_sb, in_=wproj_r)
                stage1_chunk(2)
                for r in range(NR):
                    emit_unit(1, r)
                stage1_chunk(3)
                for r in range(NR):
                    emit_unit(2, r)
                for r in range(NR):
                    emit_unit(3, r)
                flush(unit)

            if loop == 1:
                body()
            else:
                with tc.For_i(
                    0, loop, 1,
                    hint_engines=(
                        mybir.EngineType.PE,
                        mybir.EngineType.Activation,
                        mybir.EngineType.DVE,
                        mybir.EngineType.SP,
                        mybir.EngineType.Pool,
                    ),
                ):
                    body()

    nc.finalize()
    return nc


def _shard_inputs(x, Wqkv, bqkv, Wproj):
    """Build the 8 per-core input maps."""
    x = np.asarray(x, dtype=np.float32)
    Wqkv = np.asarray(Wqkv, dtype=np.float32)
    bqkv = np.asarray(bqkv, dtype=np.float32)
    Wproj = np.asarray(Wproj, dtype=np.float32)

    in_maps = []
    for c in range(NCORES):
        b = c // GROUPS
        g = c % GROUPS
        cols = slice(g * CQ, (g + 1) * CQ)
        wq = Wqkv[:, cols] * SCALE
        wk = Wqkv[:, EMB:][:, cols]
        wv = Wqkv[:, 2 * EMB:][:, cols]
        w_c = np.ascontiguousarray(
            np.concatenate([wq, wk, wv], axis=1).astype(np.float16)
        )
        bq = bqkv[cols] * SCALE
        bk = bqkv[EMB:][cols]
        b_c = np.concatenate([bq, bk])  # [512]; v bias handled on host
        b_c = np.ascontiguousarray(b_c.reshape(4, 128).T)  # [128, 4]
        wp_c = np.ascontiguousarray(Wproj[cols, :].astype(np.float16))
        xT_c = np.ascontiguousarray(x[b].T.astype(np.float16))  # [1024, 2048]
        in_maps.append({"xT": xT_c, "wqkv": w_c, "bqkv": b_c, "wproj": wp_c})
    return in_maps


def run(inputs, trace=False, **kwargs):
    """Build (cached), run on 8 cores, return (full_output, BassKernelResults)."""
    from concourse.bass_utils import run_bass_kernel_spmd

    if _compiled.get(1) is None:
        _compiled[1] = _build()
    in_maps = _shard_inputs(
        inputs["x"], inputs["Wqkv"], inputs["bqkv"], inputs["Wproj"]
    )
    res = run_bass_kernel_spmd(
        _compiled[1], in_maps, core_ids=list(range(NCORES)), trace=trace, **kwargs
    )
    partials = np.stack([res.results[c]["out"] for c in range(NCORES)])  # [8,T,EMB]
    bqkv_f = np.asarray(inputs["bqkv"], dtype=np.float64)
    wproj_f = np.asarray(inputs["Wproj"], dtype=np.float64)
    bias = (
        np.asarray(inputs["bproj"], dtype=np.float64)
        + bqkv_f[2 * EMB :] @ wproj_f
    ).astype(np.float32)
    full = np.stack(
        [partials[b * GROUPS : (b + 1) * GROUPS].sum(axis=0) for b in range(B)]
    ) + bias
    return full.astype(np.float32), res


def kernel(**inputs):
    out, _ = run(inputs)
    return out


# revision 7
# speedup vs baseline: 1.4535x; 1.2767x over previous
"""Causal self-attention (B=2, T=2048, EMB=1024, 16 heads) on 8 TRN2 NeuronCores.

Sharding: core c handles batch c//4 and heads [4*(c%4), 4*(c%4)+4).
 - Wqkv is split column-wise per head group (q part pre-scaled by 1/sqrt(hd)),
 - Wproj is split row-wise per head group,
 - each core emits a partial [2048, 1024] projection output,
 - host sums the 4 partials per batch and adds bproj + bv@Wproj
   (softmax rows sum to 1, so the v bias contributes a constant row vector
   that the host can add; the device kernel drops bv entirely).

Device kernel (per core, SPMD):
 - host supplies x^T so both qkv matmul operands have the contraction on
   partitions; qkT is produced directly in [qkv_col, token] (transposed)
   layout. v is produced token-major directly (lhsT = x^T token tile), no
   PE transposes needed, and lands next to a ones column (row-sum trick
   for the softmax denominator).
 - attention runs in the S^T = (K Q^T) layout, chunk-major. Within a chunk
   the two head-pair units are interleaved j-tile by j-tile so four
   independent S->exp chains are always in flight; PV matmuls trail one
   block of 4 j-tiles as contiguous per-head accumulation chains (avoids
   the HW per-matmul PSUM-group-switch cost).
 - at chunk end each [65, 512] PV accumulator (row 64 = softmax denominator
   via the ones column) is evacuated to SBUF, freeing all four PSUM banks
   for the next chunk while the normalize/projection epilogue is deferred.
 - deferred work (next chunk's stage-1 pieces first, then epilogue
   normalize + projection pieces) drains one piece per j-step through the
   attention loops, keeping PE busy during exp waits.
 - softmax denominator DMAs (partition spread for the 128-lane reciprocal)
   ride the gpsimd/Pool SWDGE queue so they never block the activation queue.

All matmul operands are f16 (~1e-4 rel err); PSUM accumulation in f32.
"""
import sys

sys.path.insert(0, "/opt/trn_rl_repo")

import numpy as np

B = 2
T = 2048
EMB = 1024
HEADS = 16
HD = EMB // HEADS  # 64
NCORES = 8
GROUPS = 4                 # head groups (cores per batch)
HPC = HEADS // GROUPS      # 4 heads per core
CQ = HPC * HD              # 256 q (or k or v) columns per core
KT = EMB // 128            # 8 contraction tiles
TCH = 512                  # token chunk
NCH = T // TCH             # 4 chunks
NTT = T // 128             # 16 token tiles
NR = CQ // 128             # 2 head-dim row tiles (= head pairs)
SCALE = HD ** -0.5

_compiled = {}
ABLATE = None  # None | 's1' (stage1 only) | 's12' (no projection)


def _build(loop=1):
    import concourse.bass as bass
    import concourse.tile as tile
    from concourse import bacc, mybir

    F32 = mybir.dt.float32
    F16 = mybir.dt.float16
    AF = mybir.ActivationFunctionType

    nc = bacc.Bacc(None, target_bir_lowering=False)
    xT = nc.dram_tensor("xT", [EMB, T], F16, kind="ExternalInput")
    wqkv = nc.dram_tensor("wqkv", [EMB, 3 * CQ], F16, kind="ExternalInput")
    bqkv = nc.dram_tensor("bqkv", [128, 4], F32, kind="ExternalInput")
    wproj = nc.dram_tensor("wproj", [CQ, EMB], F16, kind="ExternalInput")
    out = nc.dram_tensor("out", [T, EMB], F32, kind="ExternalOutput")

    xT_r = xT.rearrange("(kt p) t -> p kt t", p=128)
    wqkv_r = wqkv.rearrange("(kt p) c -> p kt c", p=128)
    wproj_r = wproj.rearrange("(r p) e -> p r e", p=128)

    with tile.TileContext(nc) as tc:
        with (
            tc.tile_pool(name="const", bufs=1) as const,
            tc.tile_pool(name="qk", bufs=1) as qkp,
            tc.tile_pool(name="xt", bufs=3) as xtp,
            tc.tile_pool(name="pt", bufs=20) as ptp,
            tc.tile_pool(name="oh", bufs=1) as ohp,
            tc.tile_pool(name="csb", bufs=8) as csbp,
            tc.tile_pool(name="den", bufs=5) as denp,
            tc.tile_pool(name="osb", bufs=3) as osbp,
            tc.tile_pool(name="ps", bufs=4, space="PSUM") as psS,
            tc.tile_pool(name="psO", bufs=4, space="PSUM") as psO,
        ):
            # ---- constants ----
            # weights on the scalar HWDGE queue, per k-tile, so the sync
            # queue's xt chunk loads run in parallel and matmuls start early
            bias_sb = const.tile([128, 4], F32)
            nc.scalar.dma_start(out=bias_sb, in_=bqkv[:, :])
            w_sb = const.tile([128, KT, 3 * CQ], F16)
            for kt in range(KT):
                nc.scalar.dma_start(
                    out=w_sb[:, kt, 0:CQ], in_=wqkv_r[:, kt, 0:CQ]
                )
            for cp in range(1, 3):
                nc.scalar.dma_start(
                    out=w_sb[:, :, cp * CQ : (cp + 1) * CQ],
                    in_=wqkv_r[:, :, cp * CQ : (cp + 1) * CQ],
                )
            # stage-3 weights loaded inside body() after the xt chunks
            wp_sb = const.tile([128, NR, EMB], F16)
            tri_f = const.tile([128, 128], F32)
            nc.gpsimd.memset(tri_f, 1.0)
            # keep where i(free) >= j(partition): -j + i >= 0
            nc.gpsimd.affine_select(
                out=tri_f, in_=tri_f,
                compare_op=mybir.AluOpType.is_ge,
                fill=0.0, base=0,
                pattern=[[1, 128]], channel_multiplier=-1,
            )
            tri = const.tile([128, 128], F16)
            nc.vector.tensor_copy(tri, tri_f)
            ones_f = const.tile([128, 64], F32)
            nc.vector.memset(ones_f, 1.0)
            ones64 = const.tile([1, 64], F16)
            nc.vector.tensor_copy(ones64, ones_f[0:1, :])
            # v in token-major, per (token_tile, head): 64 cols + ones col
            v_sb = const.tile([128, NTT, HPC, HD + 1], F16)
            nc.vector.tensor_copy(
                out=v_sb[:, :, :, HD : HD + 1],
                in_=ones_f.rearrange("p (a b c) -> p a b c", a=NTT, b=HPC),
            )
            qkT_sb = qkp.tile([128, 4, T], F16)
            ohT = ohp.tile([128, NR, T], F16)

            def body():
                # two-priority deferred-work queues: (chunk, fn) stage-1
                # pieces first, then epilogue pieces
                q_s1 = []
                q_epi = []

                def emit_filler(n=1):
                    for _ in range(n):
                        if q_s1:
                            q_s1.pop(0)[1]()
                        elif q_epi:
                            q_epi.pop(0)()
                        else:
                            return

                def force_s1(upto_ch):
                    while q_s1 and q_s1[0][0] <= upto_ch:
                        q_s1.pop(0)[1]()

                # ---- stage 1: qkv projection ----
                # qkT_sb[:, cb, t]: cb 0,1 = q col-tiles, 2,3 = k col-tiles
                # (transposed layout); v goes token-major straight into v_sb
                def stage1_chunk(ch, inline):
                    xt = xtp.tile([128, KT, TCH], F16)
                    if ch == 0:
                        for kt in range(KT):
                            nc.sync.dma_start(
                                out=xt[:, kt, :],
                                in_=xT_r[:, kt, ch * TCH : (ch + 1) * TCH],
                            )
                    else:
                        nc.sync.dma_start(
                            out=xt, in_=xT_r[:, :, ch * TCH : (ch + 1) * TCH]
                        )

                    def make_qk(cb):
                        def qk_piece():
                            ps = psS.tile([128, TCH], mybir.dt.float32, tag="ps")
                            for kt in range(KT):
                                nc.tensor.matmul(
                                    ps,
                                    w_sb[:, kt, cb * 128 : (cb + 1) * 128],
                                    xt[:, kt, :],
                                    start=(kt == 0),
                                    stop=(kt == KT - 1),
                                )
                            nc.vector.tensor_scalar_add(
                                qkT_sb[:, cb, ch * TCH : (ch + 1) * TCH],
                                ps,
                                bias_sb[:, cb : cb + 1],
                            )
                        return qk_piece

                    def make_v(s):
                        def v_piece():
                            tt = ch * (TCH // 128) + s
                            psv = psS.tile([128, CQ], mybir.dt.float32, tag="ps")
                            for kt in range(KT):
                                nc.tensor.matmul(
                                    psv,
                                    xt[:, kt, s * 128 : (s + 1) * 128],
                                    w_sb[:, kt, 2 * CQ : 3 * CQ],
                                    start=(kt == 0),
                                    stop=(kt == KT - 1),
                                )
                            nc.vector.tensor_copy(
                                v_sb[:, tt, :, 0:HD],
                                psv.rearrange("p (h d) -> p h d", h=HPC),
                            )
                        return v_piece

                    pieces = [make_qk(cb) for cb in range(4)]
                    pieces += [make_v(s) for s in range(TCH // 128)]
                    if inline:
                        for p in pieces:
                            p()
                    else:
                        q_s1.extend((ch, p) for p in pieces)

                def make_partB_rp(rec_rows, holder):
                    def partB_rp():
                        # packed reciprocal broadcast: head-even -> psum rows
                        # 0:64, head-odd -> rows 64:128
                        rp = psS.tile([128, TCH], mybir.dt.float32, tag="ps")
                        nc.tensor.matmul(
                            rp[0:64, :], ones64, rec_rows[0], start=True, stop=True
                        )
                        nc.tensor.matmul(
                            rp[64:128, :], ones64, rec_rows[1], start=True, stop=True
                        )
                        rec_sb = denp.tile([64, 2, TCH], F32, tag="rec_sb")
                        nc.vector.tensor_copy(rec_sb[:, 0, :], rp[0:64, :])
                        nc.vector.tensor_copy(rec_sb[:, 1, :], rp[64:128, :])
                        holder.append(rec_sb)
                    return partB_rp

                def make_partB_mul(cs2, holder, r, base):
                    def partB_mul():
                        rec_sb = holder[0]
                        nc.vector.tensor_mul(
                            ohT[0:64, r, base : base + TCH],
                            cs2[0][0:64, :],
                            rec_sb[:, 0, :],
                        )
                        nc.vector.tensor_mul(
                            ohT[64:128, r, base : base + TCH],
                            cs2[1][0:64, :],
                            rec_sb[:, 1, :],
                        )
                    return partB_mul

                def make_proj(tt, nn):
                    def proj_piece():
                        pp = psS.tile([128, TCH], mybir.dt.float32, tag="ps")
                        for r2 in range(NR):
                            nc.tensor.matmul(
                                pp,
                                ohT[:, r2, tt * 128 : (tt + 1) * 128],
                                wp_sb[:, r2, nn * 512 : (nn + 1) * 512],
                                start=(r2 == 0),
                                stop=(r2 == NR - 1),
                            )
                        osb = osbp.tile([128, 512], F32)
                        nc.any.tensor_copy(osb, pp)
                        nc.sync.dma_start(
                            out=out[
                                tt * 128 : (tt + 1) * 128,
                                nn * 512 : (nn + 1) * 512,
                            ],
                            in_=osb,
                        )
                    return proj_piece

                def emit_chunk(cc):
                    base = cc * TCH
                    jmax = 4 * cc + 3
                    diag = [j for j in range(4 * cc, jmax + 1) if j != 0]
                    rest = [j for j in range(1, 4 * cc)]
                    order = [0] + diag + rest
                    jlast = order[-1]
                    psC = [
                        [
                            psO.tile(
                                [65, TCH], mybir.dt.float32, tag="psO",
                                name=f"psC_{cc}_{_r}_{_hh}",
                            )
                            for _hh in range(2)
                        ]
                        for _r in range(NR)
                    ]
                    blks = [[] for _ in range(NR)]

                    def pv_chain(r, blk):
                        for hh in range(2):
                            for jt, pts, lo, hi in blk:
                                nc.tensor.matmul(
                                    psC[r][hh][:, lo - base : hi - base],
                                    v_sb[:, jt, 2 * r + hh, :],
                                    pts[hh][:, 0 : hi - lo],
                                    start=(jt == 0),
                                    stop=(jt == jlast),
                                    skip_group_check=(jt != 0),
                                )

                    for pos, jt in enumerate(order):
                        i0 = 128 * jt
                        lo = max(base, i0)
                        hi = base + TCH
                        w = hi - lo
                        for r in range(NR):
                            pts = []
                            for hh in range(2):
                                po = 64 * hh
                                sp = psS.tile(
                                    [128, TCH], mybir.dt.float32, tag="ps"
                                )
                                nc.tensor.matmul(
                                    sp[:, 0:w],
                                    qkT_sb[po : po + 64, 2 + r, i0 : i0 + 128],
                                    qkT_sb[po : po + 64, r, lo:hi],
                                    start=True,
                                    stop=True,
                                )
                                pt = ptp.tile([128, TCH], F16)
                                nc.scalar.activation(
                                    pt[:, 0:w], sp[:, 0:w], AF.Exp
                                )
                                if i0 >= base:  # diagonal block: causal mask
                                    nc.vector.tensor_mul(
                                        pt[:, 0:128], pt[:, 0:128], tri
                                    )
                                pts.append(pt)
                            blks[r].append((jt, pts, lo, hi))
                            if len(blks[r]) == 4 and pos < len(order) - 1:
                                pv_chain(r, blks[r])
                                blks[r] = []
                        emit_filler(1)
                    for r in range(NR):
                        pv_chain(r, blks[r])
                    # evacuate accumulators to SBUF (frees PSUM), then the
                    # denominator chains (no PE work; DMAs on the Pool queue)
                    for r in range(NR):
                        cs2 = []
                        rec_rows = []
                        for hh in range(2):
                            csb = csbp.tile([65, TCH], F32)
                            nc.vector.tensor_copy(csb, psC[r][hh])
                            cs2.append(csb)
                            den128 = denp.tile([128, TCH // 128], F32, tag="den128")
                            nc.gpsimd.dma_start(out=den128, in_=csb[64:65, :])
                            rec128 = denp.tile([128, TCH // 128], F32, tag="rec128")
                            nc.vector.reciprocal(rec128, den128)
                            rec16 = denp.tile([128, TCH // 128], F16, tag="rec16")
                            nc.vector.tensor_copy(rec16, rec128)
                            rec_row = denp.tile([1, TCH], F16, tag="rec_row")
                            nc.gpsimd.dma_start(out=rec_row, in_=rec16)
                            rec_rows.append(rec_row)
                        holder = []
                        q_epi.append(make_partB_rp(rec_rows, holder))
                        q_epi.append(make_partB_mul(cs2, holder, r, base))
                    if ABLATE != "s12":
                        for tt in range(4 * cc, 4 * cc + 4):
                            for nn in range(EMB // 512):
                                q_epi.append(make_proj(tt, nn))

                # ---- emission schedule ----
                stage1_chunk(0, inline=True)
                if ABLATE == "s1":
                    for ch in range(1, NCH):
                        stage1_chunk(ch, inline=True)
                    return
                stage1_chunk(1, inline=False)
                nc.sync.dma_start(out=wp_sb, in_=wproj_r)
                emit_chunk(0)
                stage1_chunk(2, inline=False)
                force_s1(1)
                emit_chunk(1)
                stage1_chunk(3, inline=False)
                force_s1(2)
                emit_chunk(2)
                force_s1(3)
                emit_chunk(3)
                while q_s1 or q_epi:
                    emit_filler(1)

            if loop == 1:
                body()
            else:
                with tc.For_i(
                    0, loop, 1,
                    hint_engines=(
                        mybir.EngineType.PE,
                        mybir.EngineType.Activation,
                        mybir.EngineType.DVE,
                        mybir.EngineType.SP,
                        mybir.EngineType.Pool,
                    ),
                ):
                    body()

    nc.finalize()
    return nc


def _shard_inputs(x, Wqkv, bqkv, Wproj):
    """Build the 8 per-core input maps."""
    x = np.asarray(x, dtype=np.float32)
    Wqkv = np.asarray(Wqkv, dtype=np.float32)
    bqkv = np.asarray(bqkv, dtype=np.float32)
    Wproj = np.asarray(Wproj, dtype=np.float32)

    in_maps = []
    for c in range(NCORES):
        b = c // GROUPS
        g = c % GROUPS
        cols = slice(g * CQ, (g + 1) * CQ)
        wq = Wqkv[:, cols] * SCALE
        wk = Wqkv[:, EMB:][:, cols]
        wv = Wqkv[:, 2 * EMB:][:, cols]
        w_c = np.ascontiguousarray(
            np.concatenate([wq, wk, wv], axis=1).astype(np.float16)
        )
        bq = bqkv[cols] * SCALE
        bk = bqkv[EMB:][cols]
        b_c = np.concatenate([bq, bk])  # [512]; v bias handled on host
        b_c = np.ascontiguousarray(b_c.reshape(4, 128).T)  # [128, 4]
        wp_c = np.ascontiguousarray(Wproj[cols, :].astype(np.float16))
        xT_c = np.ascontiguousarray(x[b].T.astype(np.float16))  # [1024, 2048]
        in_maps.append({"xT": xT_c, "wqkv": w_c, "bqkv": b_c, "wproj": wp_c})
    return in_maps


def run(inputs, trace=False, **kwargs):
    """Build (cached), run on 8 cores, return (full_output, BassKernelResults)."""
    from concourse.bass_utils import run_bass_kernel_spmd

    if _compiled.get(1) is None:
        _compiled[1] = _build()
    in_maps = _shard_inputs(
        inputs["x"], inputs["Wqkv"], inputs["bqkv"], inputs["Wproj"]
    )
    res = run_bass_kernel_spmd(
        _compiled[1], in_maps, core_ids=list(range(NCORES)), trace=trace, **kwargs
    )
    partials = np.stack([res.results[c]["out"] for c in range(NCORES)])  # [8,T,EMB]
    bqkv_f = np.asarray(inputs["bqkv"], dtype=np.float64)
    wproj_f = np.asarray(inputs["Wproj"], dtype=np.float64)
    bias = (
        np.asarray(inputs["bproj"], dtype=np.float64)
        + bqkv_f[2 * EMB :] @ wproj_f
    ).astype(np.float32)
    full = np.stack(
        [partials[b * GROUPS : (b + 1) * GROUPS].sum(axis=0) for b in range(B)]
    ) + bias
    return full.astype(np.float32), res


def kernel(**inputs):
    out, _ = run(inputs)
    return out
